# revision 29
# baseline (speedup 1.0000x reference)
"""Trainium2 Bass kernel for BlockDiagonalAggregator (moe_routing).

Computes, for each batch row b:
    logit[b,k] = dot(keys[sigma[b,k]], h[b,k,:])   (masked where sigma==64)
    alpha      = softmax_k(logit)
    out[b,:]   = sum_k alpha[b,k] * h[b,k,:]

Distribution: data-parallel over B across 8 NeuronCores (512 rows each),
keys replicated, no collectives (per the data-parallel sharding hint).

Per-core algorithm (single streaming pass over h, all 16-bit traffic):
  - h is cast to bf16 host-side and pre-shuffled to (macro, partition,
    chunk, d); this halves HBM traffic vs fp32, the dominant cost.
  - chunk = 128 (b,k)-slots = 2 batch rows (K=64); macro = 16 chunks.
  - w gather via one-hot matmul on PE: w = ohT.T @ keys (bf16, one-hot
    built host-side from sigma), fp32 result in PSUM.
  - ACT (otherwise idle) copies w PSUM -> SBUF bf16 so the logit dot can
    run on DVE in 2x_1P mode (both operands 16-bit SBUF): one fused
    scalar_tensor_tensor (out=(h*1)*w, accum=sum_d) per chunk.
  - exp via degree-8 polynomial evaluated on DVE over a whole macro's
    logits [128,16] at once (|logit| <= ~1.1 since keys std 0.01; fitted
    on [-2.5,2.5], rel err 1.4e-3 at the edge, ~1e-5 in-range). Horner in
    fused (t + c)*x form = 8 cheap DVE ops per macro, replacing 512
    FD=1 ACT activations. The final Horner step fuses the unassigned-slot
    mask multiply AND scatters into the block-diagonal E cells via a
    stride-34 diagonal access pattern.
  - E is one persistent [128, 512] bf16 tile per macro parity (zeroed
    once at start); chunk c's pooling stationary is E[:, 32c:32c+32] with
    its e-values at global cols 34c (rows 0:64) / 34c+1 (rows 64:128),
    i.e. local cols 2c, 2c+1 -> macro output rows 2c, 2c+1.
  - PE pooling: pool += E_c.T @ h_c and esum += E_c.T @ ones, accumulated
    in PSUM over a macro's 16 chunks -> (32 rows, 512) + (32, 2), all
    bf16 operands (1 cycle/row).
  - out = pool * (1/esum) on DVE, DMA out fp32.
"""

import numpy as np
import ml_dtypes

# Problem constants (hardcoded: kernel.py must be self-contained)
B, K, D = 4096, 64, 512
N_AGENTS = 64
N_CORES = 8
B_CORE = B // N_CORES            # 512
BK_CORE = B_CORE * K             # 32768
CHUNK = 128                      # bk-slots per chunk (= 2 batch rows)
CHUNKS_PER_MACRO = 16            # chunks per macro (= 32 batch rows)
MACRO_BK = CHUNK * CHUNKS_PER_MACRO   # 2048

# exp(x) ~= sum_k EXP_POLY[k] x^k, Chebyshev fit on [-2.5, 2.5] (deg 8).
EXP_POLY = [1.000004066314573, 0.9998755097544132, 0.4999644070638146,
            0.16695633150372563, 0.04171564062371739, 0.008155067657787365,
            0.00136567602494669, 0.00023806040948123335,
            2.9200850006302826e-05]

_prog_cache = {}


KEY_SCALE = 256.0   # fp8 keys are scaled by this; exp() divides it back out


def _build_program(n_macros: int, repeat: int = 1, dma_only: bool = False,
                   n_f32_dots: int = 0, exp_mode: str = "poly",
                   norm_mode: str = "dve", pe_pairs: bool = False,
                   fp8_oh: bool = False):
    """Build the SPMD single-core Bass program for a shard of
    n_macros * MACRO_BK (b,k)-slots.

    n_f32_dots: of the 16 chunks per macro, this many use the direct
      fp32-PSUM stt dot (no ACT copy); the rest use ACT copy + bf16 stt.
    exp_mode: "poly" (Horner on DVE) or "act" (batched ACT Exp + DVE
      mask-scatter).
    norm_mode: "dve" (tensor_scalar_mul) or "act" (ACT Copy with scale).
    """
    import contextlib
    import concourse.bacc as bacc
    import concourse.tile as tile
    import concourse.mybir as mybir

    f32 = mybir.dt.float32
    bf16 = mybir.dt.bfloat16
    fp8 = mybir.dt.float8e4
    ohdt = fp8 if fp8_oh else bf16
    ALU = mybir.AluOpType
    AF = mybir.ActivationFunctionType

    bk = n_macros * MACRO_BK
    b_rows = bk // K
    RPM = MACRO_BK // K   # 32 output rows per macro
    CPM = CHUNKS_PER_MACRO
    ECOLS = 2 * CPM * CPM // 1  # 512 (32 stationary cols x 16 chunks)

    nc = bacc.Bacc("TRN2", target_bir_lowering=False, debug=False,
                   num_devices=N_CORES)

    h_d = nc.dram_tensor("h", [n_macros, CHUNK, CPM * D], bf16,
                         kind="ExternalInput").ap()
    if pe_pairs:
        # chunk pair (2i, 2i+1) one-hots packed into partitions 0:64 / 64:128
        oh_d = nc.dram_tensor("oh", [n_macros, CHUNK, CPM // 2, CHUNK],
                              ohdt, kind="ExternalInput").ap()
        keys_shape = [CHUNK, D]   # keys duplicated into both halves
    else:
        oh_d = nc.dram_tensor("oh", [n_macros, N_AGENTS, CPM, CHUNK],
                              ohdt, kind="ExternalInput").ap()
        keys_shape = [N_AGENTS, D]
    mask_d = nc.dram_tensor("mask", [CHUNK, n_macros * CPM], bf16,
                            kind="ExternalInput").ap()
    keys_d = nc.dram_tensor("keys", keys_shape, ohdt,
                            kind="ExternalInput").ap()
    ones_d = nc.dram_tensor("ones", [CHUNK, 2], bf16,
                            kind="ExternalInput").ap()
    ez_d = nc.dram_tensor("ez", [CHUNK, ECOLS], bf16,
                          kind="ExternalInput").ap()
    out_d = nc.dram_tensor("out", [b_rows, D], f32, kind="ExternalOutput").ap()

    with tile.TileContext(nc) as tc:
        with (
            tc.tile_pool(name="const", bufs=1) as const_pool,
            tc.tile_pool(name="h", bufs=3) as h_pool,
            tc.tile_pool(name="oh", bufs=2) as oh_pool,
            tc.tile_pool(name="wsb", bufs=3) as wsb_pool,
            tc.tile_pool(name="tmp", bufs=2) as tmp_pool,
            tc.tile_pool(name="logit", bufs=2) as logit_pool,
            tc.tile_pool(name="poly", bufs=2) as poly_pool,
            tc.tile_pool(name="outp", bufs=2) as out_pool,
            tc.tile_pool(name="recip", bufs=2) as recip_pool,
            tc.tile_pool(name="psw", bufs=(2 if pe_pairs else 3),
                         space="PSUM") as psw,
            tc.tile_pool(name="psp", bufs=2, space="PSUM") as psp,
            tc.tile_pool(name="pse", bufs=2, space="PSUM") as pse,
        ):
            keys_t = const_pool.tile(keys_shape, ohdt)
            nc.sync.dma_start(keys_t[:], keys_d[:])
            ones_t = const_pool.tile([CHUNK, 2], bf16)
            nc.sync.dma_start(ones_t[:], ones_d[:])
            mask_t = const_pool.tile([CHUNK, n_macros * CPM], bf16)
            nc.scalar.dma_start(mask_t[:], mask_d[:])

            # persistent per-parity E tiles; nonzero cells rewritten by the
            # poly-exp scatter each macro, the rest stay zero forever
            E_tiles = []
            for par in range(2):
                e = const_pool.tile([CHUNK, ECOLS], bf16, tag=f"ebig_{par}")
                nc.sync.dma_start(e[:], ez_d[:])
                E_tiles.append(e)

            hd3 = h_d.rearrange("m p (c d) -> m p c d", d=D)
            half = CHUNK // 2  # 64 = K

            def emit_dma(m):
                """issue h + one-hot DMAs for macro m (prefetched deep)."""
                h_t = h_pool.tile([CHUNK, CPM, D], bf16, name="h_t")
                hc = CPM // 2
                nc.sync.dma_start(h_t[:, 0:hc, :], hd3[m][:, 0:hc, :])
                nc.sync.dma_start(h_t[:, hc:, :], hd3[m][:, hc:, :])
                if pe_pairs:
                    oh_t = oh_pool.tile([CHUNK, CPM // 2, CHUNK], ohdt,
                                        name="oh_t")
                else:
                    oh_t = oh_pool.tile([N_AGENTS, CPM, CHUNK], ohdt,
                                        name="oh_t")
                nc.gpsimd.dma_start(oh_t[:], oh_d[m])
                return h_t, oh_t

            def emit_head(m, h_t, oh_t):
                """w-gather + ACT copy + dots for macro m."""
                logit_t = logit_pool.tile([CHUNK, CPM], f32,
                                          name="logit_t")
                if pe_pairs:
                    for i in range(CPM // 2):
                        # two row-group matmuls run concurrently on PE
                        w_ps2 = psw.tile([CHUNK, 2, D], f32)
                        nc.tensor.matmul(w_ps2[:, 0, :], oh_t[0:half, i, :],
                                         keys_t[0:half, :],
                                         start=True, stop=True)
                        nc.tensor.matmul(w_ps2[:, 1, :],
                                         oh_t[half:CHUNK, i, :],
                                         keys_t[half:CHUNK, :],
                                         start=True, stop=True)
                        if 2 * i + 1 < n_f32_dots:
                            for j in range(2):
                                c = 2 * i + j
                                tmp_f = tmp_pool.tile([CHUNK, D], f32,
                                                      tag="tmpf")
                                nc.vector.scalar_tensor_tensor(
                                    tmp_f[:], h_t[:, c, :], 1.0,
                                    w_ps2[:, j, :],
                                    op0=ALU.mult, op1=ALU.mult,
                                    accum_out=logit_t[:, c:c + 1])
                        else:
                            # one ACT copy for both chunks of the pair
                            w_sb = wsb_pool.tile([CHUNK, 2, D], bf16)
                            nc.scalar.copy(w_sb[:], w_ps2[:])
                            for j in range(2):
                                c = 2 * i + j
                                tmp_t = tmp_pool.tile([CHUNK, D], bf16)
                                nc.vector.scalar_tensor_tensor(
                                    tmp_t[:], h_t[:, c, :], 1.0,
                                    w_sb[:, j, :],
                                    op0=ALU.mult, op1=ALU.mult,
                                    accum_out=logit_t[:, c:c + 1])
                else:
                    for c in range(CPM):
                        w_ps = psw.tile([CHUNK, D], f32)
                        nc.tensor.matmul(w_ps[:], oh_t[:, c, :], keys_t[:],
                                         start=True, stop=True)
                        if c < n_f32_dots:
                            tmp_f = tmp_pool.tile([CHUNK, D], f32,
                                                  tag="tmpf")
                            nc.vector.scalar_tensor_tensor(
                                tmp_f[:], h_t[:, c, :], 1.0, w_ps[:],
                                op0=ALU.mult, op1=ALU.mult,
                                accum_out=logit_t[:, c:c + 1])
                        else:
                            w_sb = wsb_pool.tile([CHUNK, D], bf16)
                            nc.scalar.copy(w_sb[:], w_ps[:])
                            tmp_t = tmp_pool.tile([CHUNK, D], bf16)
                            nc.vector.scalar_tensor_tensor(
                                tmp_t[:], h_t[:, c, :], 1.0, w_sb[:],
                                op0=ALU.mult, op1=ALU.mult,
                                accum_out=logit_t[:, c:c + 1])
                return logit_t

            def emit_exp(m, logit_t):
                """batched exp on ACT (issued early so the later DVE
                scatter never head-of-line blocks the DVE queue)."""
                e_t = poly_pool.tile([CHUNK, CPM], f32, tag="et")
                nc.scalar.activation(
                    e_t[:], logit_t[:], AF.Exp, bias=0.0,
                    scale=(1.0 / KEY_SCALE) if fp8_oh else 1.0)
                return e_t

            def emit_tail(m, h_t, logit_t, e_t):
                """mask-scatter + pooling + normalize + out for m."""
                E = E_tiles[m % 2]
                mc = m * CPM
                if exp_mode == "act":
                    nc.vector.scalar_tensor_tensor(
                        E[0:half, 0:ECOLS:34], e_t[0:half, :], 0.0,
                        mask_t[0:half, mc:mc + CPM],
                        op0=ALU.add, op1=ALU.mult)
                    nc.vector.scalar_tensor_tensor(
                        E[half:CHUNK, 1:ECOLS:34], e_t[half:CHUNK, :],
                        0.0, mask_t[half:CHUNK, mc:mc + CPM],
                        op0=ALU.add, op1=ALU.mult)
                else:
                    deg = len(EXP_POLY) - 1
                    t_a = poly_pool.tile([CHUNK, CPM], f32, tag="ta")
                    t_b = poly_pool.tile([CHUNK, CPM], f32, tag="tb")
                    nc.vector.tensor_scalar(t_a[:], logit_t[:],
                                            EXP_POLY[deg],
                                            None, op0=ALU.mult)
                    cur, nxt = t_a, t_b
                    for k in range(deg - 1, 0, -1):
                        nc.vector.scalar_tensor_tensor(
                            nxt[:], cur[:], EXP_POLY[k], logit_t[:],
                            op0=ALU.add, op1=ALU.mult)
                        cur, nxt = nxt, cur
                    nc.vector.scalar_tensor_tensor(
                        E[0:half, 0:ECOLS:34], cur[0:half, :], EXP_POLY[0],
                        mask_t[0:half, mc:mc + CPM],
                        op0=ALU.add, op1=ALU.mult)
                    nc.vector.scalar_tensor_tensor(
                        E[half:CHUNK, 1:ECOLS:34], cur[half:CHUNK, :],
                        EXP_POLY[0], mask_t[half:CHUNK, mc:mc + CPM],
                        op0=ALU.add, op1=ALU.mult)

                # pool += E_c.T @ h_c ; esum += E_c.T @ ones
                pool_ps = psp.tile([RPM, D], f32)
                esum_ps = pse.tile([RPM, 2], f32)
                for c in range(CPM):
                    first, last = (c == 0), (c == CPM - 1)
                    Ec = E[:, 32 * c:32 * c + 32]
                    nc.tensor.matmul(pool_ps[:], Ec, h_t[:, c, :],
                                     start=first, stop=last)
                    nc.tensor.matmul(esum_ps[:], Ec, ones_t[:],
                                     start=first, stop=last)

                recip_t = recip_pool.tile([RPM, 1], f32)
                nc.vector.reciprocal(recip_t[:], esum_ps[:, 0:1])
                out_t = out_pool.tile([RPM, D], f32)
                if norm_mode == "act":
                    nc.scalar.activation(out_t[:], pool_ps[:], AF.Copy,
                                         bias=0.0, scale=recip_t[:])
                else:
                    nc.vector.tensor_scalar_mul(out_t[:], pool_ps[:],
                                                recip_t[:])
                nc.gpsimd.dma_start(out_d[m * RPM:(m + 1) * RPM, :],
                                    out_t[:])

            rep_ctx = (tc.For_i(0, repeat, 1) if repeat > 1
                       else contextlib.nullcontext())
            with rep_ctx:
                if dma_only:
                    for m in range(n_macros):
                        h_t = h_pool.tile([CHUNK, CPM, D], bf16)
                        hc = CPM // 2
                        nc.sync.dma_start(h_t[:, 0:hc, :],
                                          hd3[m][:, 0:hc, :])
                        nc.sync.dma_start(h_t[:, hc:, :], hd3[m][:, hc:, :])
                        out_t = out_pool.tile([RPM, D], f32)
                        nc.vector.tensor_copy(out_t[:], h_t[0:RPM, 0, :])
                        nc.scalar.dma_start(
                            out_d[m * RPM:(m + 1) * RPM, :], out_t[:])
                else:
                    # software pipeline: per macro, emit exp(m) first on
                    # ACT, then macro m+1's dots, then m's scatter/pooling
                    # tail — neither the DVE nor the ACT FIFO head-of-line
                    # blocks on a cross-engine dependency
                    tiles = {0: emit_dma(0)}
                    if n_macros > 1:
                        tiles[1] = emit_dma(1)
                    pending = (tiles[0][0], emit_head(0, *tiles.pop(0)))
                    for m in range(n_macros):
                        e_t = (emit_exp(m, pending[1])
                               if exp_mode == "act" else None)
                        if m + 2 < n_macros:
                            tiles[m + 2] = emit_dma(m + 2)
                        nxt = None
                        if m + 1 < n_macros:
                            tm = tiles.pop(m + 1)
                            nxt = (tm[0], emit_head(m + 1, *tm))
                        emit_tail(m, *pending, e_t)
                        pending = nxt

    nc.compile()
    return nc


# Current best configuration (kernel() and the harness use this).
DEFAULT_CFG = dict(n_f32_dots=0, exp_mode="act", norm_mode="act",
                   pe_pairs=True, fp8_oh=True)


def get_program(n_macros: int = B_CORE * K // MACRO_BK, **cfg):
    merged = {**DEFAULT_CFG, **cfg}
    key = (n_macros, tuple(sorted(merged.items())))
    if key not in _prog_cache:
        _prog_cache[key] = _build_program(n_macros, **merged)
    return _prog_cache[key]


def prep_core_inputs(h_bk: np.ndarray, sigma_bk: np.ndarray,
                     keys_bf16: np.ndarray, pe_pairs: bool = False,
                     fp8_oh: bool = False):
    """Host-side prep of one core's input map.
    h_bk: (bk, D) float32, sigma_bk: (bk,) int."""
    bk = h_bk.shape[0]
    n_macros = bk // MACRO_BK
    CPM = CHUNKS_PER_MACRO
    sig = sigma_bk.astype(np.int64)
    ohdt = ml_dtypes.float8_e4m3 if fp8_oh else ml_dtypes.bfloat16
    # one-hot (a == sigma); sigma == N_AGENTS (unassigned) matches nothing
    oh = (sig[None, :] == np.arange(N_AGENTS, dtype=np.int64)[:, None])
    oh = oh.astype(ohdt)                        # (A, bk)
    oh = oh.reshape(N_AGENTS, n_macros, CPM, CHUNK)
    oh = np.ascontiguousarray(oh.transpose(1, 0, 2, 3))  # (m, A, c, j)
    if fp8_oh:
        keys_bf16 = (keys_bf16.astype(np.float32) * KEY_SCALE).astype(ohdt)
    else:
        keys_bf16 = keys_bf16.astype(ohdt)
    if pe_pairs:
        # pack chunk pair (2i, 2i+1) into partitions 0:64 / 64:128
        ohp = np.empty((n_macros, CHUNK, CPM // 2, CHUNK),
                       dtype=ohdt)
        ohp[:, 0:N_AGENTS] = oh[:, :, 0::2, :]
        ohp[:, N_AGENTS:] = oh[:, :, 1::2, :]
        oh = ohp
        keys_bf16 = np.concatenate([keys_bf16, keys_bf16], axis=0)

    mask = (sig < N_AGENTS)
    mask = mask.reshape(n_macros, CPM, CHUNK)
    mask = np.ascontiguousarray(mask.transpose(2, 0, 1))
    mask = mask.reshape(CHUNK, n_macros * CPM).astype(ml_dtypes.bfloat16)

    h_shuf = np.ascontiguousarray(
        h_bk.reshape(n_macros, CPM, CHUNK, D)
            .transpose(0, 2, 1, 3)).astype(ml_dtypes.bfloat16)
    h_shuf = h_shuf.reshape(n_macros, CHUNK, CPM * D)

    return {
        "h": h_shuf,
        "oh": oh,
        "mask": mask,
        "keys": keys_bf16,
        "ones": np.ones((CHUNK, 2), dtype=ml_dtypes.bfloat16),
        "ez": np.zeros((CHUNK, 512), dtype=ml_dtypes.bfloat16),
    }


LAST_EXEC_NS = None
LAST_TRACE = None


def kernel(h, keys, sigma):
    global LAST_EXEC_NS, LAST_TRACE
    from concourse.bass_utils import run_bass_kernel_spmd

    h = np.asarray(h, dtype=np.float32)
    keys = np.asarray(keys, dtype=np.float32)
    sigma = np.asarray(sigma)

    keys_bf16 = keys.astype(ml_dtypes.bfloat16)
    h2 = h.reshape(B * K, D)
    sig2 = sigma.reshape(B * K)

    in_maps = []
    for i in range(N_CORES):
        lo, hi = i * BK_CORE, (i + 1) * BK_CORE
        in_maps.append(prep_core_inputs(h2[lo:hi], sig2[lo:hi], keys_bf16,
                                        pe_pairs=DEFAULT_CFG["pe_pairs"],
                                        fp8_oh=DEFAULT_CFG["fp8_oh"]))

    nc = get_program()
    res = run_bass_kernel_spmd(nc, in_maps, list(range(N_CORES)))
    out = np.concatenate([res.results[i]["out"] for i in range(N_CORES)],
                         axis=0)
    if res.exec_time_ns is not None:
        LAST_EXEC_NS = res.exec_time_ns
        LAST_TRACE = res.instructions_and_trace
    return out.astype(np.float32)


if __name__ == "__main__":
    rng = np.random.default_rng(0)
    h = rng.standard_normal((B, K, D), dtype=np.float32)
    keys = (rng.standard_normal((N_AGENTS, D), dtype=np.float32) * 0.01)
    sigma = rng.integers(0, N_AGENTS + 1, size=(B, K)).astype(np.int32)
    out = kernel(h=h, keys=keys, sigma=sigma)
    print("out", out.shape, out.dtype, float(np.abs(out).mean()))


# revision 30
# speedup vs baseline: 1.4924x; 1.4924x over previous
"""Trainium2 Bass kernel for BlockDiagonalAggregator (moe_routing).

Computes, for each batch row b:
    logit[b,k] = dot(keys[sigma[b,k]], h[b,k,:])   (masked where sigma==64)
    alpha      = softmax_k(logit)
    out[b,:]   = sum_k alpha[b,k] * h[b,k,:]

Distribution: data-parallel over B across 8 NeuronCores (512 rows each),
keys replicated, no collectives (per the data-parallel sharding hint).

Per-core algorithm (single streaming pass over h, all 16-bit traffic):
  - h is cast to bf16 host-side and pre-shuffled to (macro, partition,
    chunk, d); this halves HBM traffic vs fp32, the dominant cost.
  - chunk = 128 (b,k)-slots = 2 batch rows (K=64); macro = 16 chunks.
  - w gather via one-hot matmul on PE: w = ohT.T @ keys (bf16, one-hot
    built host-side from sigma), fp32 result in PSUM.
  - ACT (otherwise idle) copies w PSUM -> SBUF bf16 so the logit dot can
    run on DVE in 2x_1P mode (both operands 16-bit SBUF): one fused
    scalar_tensor_tensor (out=(h*1)*w, accum=sum_d) per chunk.
  - exp via degree-8 polynomial evaluated on DVE over a whole macro's
    logits [128,16] at once (|logit| <= ~1.1 since keys std 0.01; fitted
    on [-2.5,2.5], rel err 1.4e-3 at the edge, ~1e-5 in-range). Horner in
    fused (t + c)*x form = 8 cheap DVE ops per macro, replacing 512
    FD=1 ACT activations. The final Horner step fuses the unassigned-slot
    mask multiply AND scatters into the block-diagonal E cells via a
    stride-34 diagonal access pattern.
  - E is one persistent [128, 512] bf16 tile per macro parity (zeroed
    once at start); chunk c's pooling stationary is E[:, 32c:32c+32] with
    its e-values at global cols 34c (rows 0:64) / 34c+1 (rows 64:128),
    i.e. local cols 2c, 2c+1 -> macro output rows 2c, 2c+1.
  - PE pooling: pool += E_c.T @ h_c and esum += E_c.T @ ones, accumulated
    in PSUM over a macro's 16 chunks -> (32 rows, 512) + (32, 2), all
    bf16 operands (1 cycle/row).
  - out = pool * (1/esum) on DVE, DMA out fp32.
"""

import numpy as np
import ml_dtypes

# Problem constants (hardcoded: kernel.py must be self-contained)
B, K, D = 4096, 64, 512
N_AGENTS = 64
N_CORES = 8
B_CORE = B // N_CORES            # 512
BK_CORE = B_CORE * K             # 32768
CHUNK = 128                      # bk-slots per chunk (= 2 batch rows)
CHUNKS_PER_MACRO = 16            # chunks per macro (= 32 batch rows)
MACRO_BK = CHUNK * CHUNKS_PER_MACRO   # 2048

# exp(x) ~= sum_k EXP_POLY[k] x^k, Chebyshev fit on [-2.5, 2.5] (deg 8).
EXP_POLY = [1.000004066314573, 0.9998755097544132, 0.4999644070638146,
            0.16695633150372563, 0.04171564062371739, 0.008155067657787365,
            0.00136567602494669, 0.00023806040948123335,
            2.9200850006302826e-05]

_prog_cache = {}


KEY_SCALE = 256.0   # fp8 keys are scaled by this; exp() divides it back out


def _build_program(n_macros: int, repeat: int = 1, dma_only: bool = False,
                   n_f32_dots: int = 0, exp_mode: str = "poly",
                   norm_mode: str = "dve", pe_pairs: bool = False,
                   fp8_oh: bool = False):
    """Build the SPMD single-core Bass program for a shard of
    n_macros * MACRO_BK (b,k)-slots.

    n_f32_dots: of the 16 chunks per macro, this many use the direct
      fp32-PSUM stt dot (no ACT copy); the rest use ACT copy + bf16 stt.
    exp_mode: "poly" (Horner on DVE) or "act" (batched ACT Exp + DVE
      mask-scatter).
    norm_mode: "dve" (tensor_scalar_mul) or "act" (ACT Copy with scale).
    """
    import contextlib
    import concourse.bacc as bacc
    import concourse.tile as tile
    import concourse.mybir as mybir

    f32 = mybir.dt.float32
    bf16 = mybir.dt.bfloat16
    fp8 = mybir.dt.float8e4
    ohdt = fp8 if fp8_oh else bf16
    ALU = mybir.AluOpType
    AF = mybir.ActivationFunctionType

    bk = n_macros * MACRO_BK
    b_rows = bk // K
    RPM = MACRO_BK // K   # 32 output rows per macro
    CPM = CHUNKS_PER_MACRO
    ECOLS = 2 * CPM * CPM // 1  # 512 (32 stationary cols x 16 chunks)

    nc = bacc.Bacc("TRN2", target_bir_lowering=False, debug=False,
                   num_devices=N_CORES)

    h_d = nc.dram_tensor("h", [n_macros, CHUNK, CPM * D], bf16,
                         kind="ExternalInput").ap()
    if pe_pairs:
        # chunk pair (2i, 2i+1) one-hots packed into partitions 0:64 / 64:128
        oh_d = nc.dram_tensor("oh", [n_macros, CHUNK, CPM // 2, CHUNK],
                              ohdt, kind="ExternalInput").ap()
        keys_shape = [CHUNK, D]   # keys duplicated into both halves
    else:
        oh_d = nc.dram_tensor("oh", [n_macros, N_AGENTS, CPM, CHUNK],
                              ohdt, kind="ExternalInput").ap()
        keys_shape = [N_AGENTS, D]
    mask_d = nc.dram_tensor("mask", [CHUNK, n_macros * CPM], bf16,
                            kind="ExternalInput").ap()
    keys_d = nc.dram_tensor("keys", keys_shape, ohdt,
                            kind="ExternalInput").ap()
    ones_d = nc.dram_tensor("ones", [CHUNK, 2], bf16,
                            kind="ExternalInput").ap()
    ez_d = nc.dram_tensor("ez", [CHUNK, ECOLS], bf16,
                          kind="ExternalInput").ap()
    out_d = nc.dram_tensor("out", [b_rows, D], f32, kind="ExternalOutput").ap()

    with tile.TileContext(nc) as tc:
        with (
            tc.tile_pool(name="const", bufs=1) as const_pool,
            tc.tile_pool(name="h", bufs=3) as h_pool,
            tc.tile_pool(name="oh", bufs=2) as oh_pool,
            tc.tile_pool(name="wsb", bufs=3) as wsb_pool,
            tc.tile_pool(name="tmp", bufs=2) as tmp_pool,
            tc.tile_pool(name="logit", bufs=2) as logit_pool,
            tc.tile_pool(name="poly", bufs=2) as poly_pool,
            tc.tile_pool(name="outp", bufs=2) as out_pool,
            tc.tile_pool(name="recip", bufs=2) as recip_pool,
            tc.tile_pool(name="psw", bufs=(2 if pe_pairs else 3),
                         space="PSUM") as psw,
            tc.tile_pool(name="psp", bufs=2, space="PSUM") as psp,
            tc.tile_pool(name="pse", bufs=2, space="PSUM") as pse,
        ):
            keys_t = const_pool.tile(keys_shape, ohdt)
            nc.sync.dma_start(keys_t[:], keys_d[:])
            ones_t = const_pool.tile([CHUNK, 2], bf16)
            nc.sync.dma_start(ones_t[:], ones_d[:])
            mask_t = const_pool.tile([CHUNK, n_macros * CPM], bf16)
            nc.scalar.dma_start(mask_t[:], mask_d[:])

            # persistent per-parity E tiles; nonzero cells rewritten by the
            # poly-exp scatter each macro, the rest stay zero forever
            E_tiles = []
            for par in range(2):
                e = const_pool.tile([CHUNK, ECOLS], bf16, tag=f"ebig_{par}")
                nc.sync.dma_start(e[:], ez_d[:])
                E_tiles.append(e)

            hd3 = h_d.rearrange("m p (c d) -> m p c d", d=D)
            half = CHUNK // 2  # 64 = K

            def emit_dma(m):
                """issue h + one-hot DMAs for macro m (prefetched deep)."""
                h_t = h_pool.tile([CHUNK, CPM, D], bf16, name="h_t")
                hc = CPM // 2
                nc.sync.dma_start(h_t[:, 0:hc, :], hd3[m][:, 0:hc, :])
                nc.sync.dma_start(h_t[:, hc:, :], hd3[m][:, hc:, :])
                if pe_pairs:
                    oh_t = oh_pool.tile([CHUNK, CPM // 2, CHUNK], ohdt,
                                        name="oh_t")
                else:
                    oh_t = oh_pool.tile([N_AGENTS, CPM, CHUNK], ohdt,
                                        name="oh_t")
                nc.gpsimd.dma_start(oh_t[:], oh_d[m])
                return h_t, oh_t

            def emit_head(m, h_t, oh_t):
                """w-gather + ACT copy + dots for macro m."""
                logit_t = logit_pool.tile([CHUNK, CPM], f32,
                                          name="logit_t")
                if pe_pairs:
                    for i in range(CPM // 2):
                        # two row-group matmuls run concurrently on PE
                        w_ps2 = psw.tile([CHUNK, 2, D], f32)
                        nc.tensor.matmul(w_ps2[:, 0, :], oh_t[0:half, i, :],
                                         keys_t[0:half, :],
                                         start=True, stop=True)
                        nc.tensor.matmul(w_ps2[:, 1, :],
                                         oh_t[half:CHUNK, i, :],
                                         keys_t[half:CHUNK, :],
                                         start=True, stop=True)
                        if 2 * i + 1 < n_f32_dots:
                            for j in range(2):
                                c = 2 * i + j
                                tmp_f = tmp_pool.tile([CHUNK, D], f32,
                                                      tag="tmpf")
                                nc.vector.scalar_tensor_tensor(
                                    tmp_f[:], h_t[:, c, :], 1.0,
                                    w_ps2[:, j, :],
                                    op0=ALU.mult, op1=ALU.mult,
                                    accum_out=logit_t[:, c:c + 1])
                        else:
                            # one ACT copy for both chunks of the pair
                            w_sb = wsb_pool.tile([CHUNK, 2, D], bf16)
                            nc.scalar.copy(w_sb[:], w_ps2[:])
                            for j in range(2):
                                c = 2 * i + j
                                tmp_t = tmp_pool.tile([CHUNK, D], bf16)
                                nc.vector.scalar_tensor_tensor(
                                    tmp_t[:], h_t[:, c, :], 1.0,
                                    w_sb[:, j, :],
                                    op0=ALU.mult, op1=ALU.mult,
                                    accum_out=logit_t[:, c:c + 1])
                else:
                    for c in range(CPM):
                        w_ps = psw.tile([CHUNK, D], f32)
                        nc.tensor.matmul(w_ps[:], oh_t[:, c, :], keys_t[:],
                                         start=True, stop=True)
                        if c < n_f32_dots:
                            tmp_f = tmp_pool.tile([CHUNK, D], f32,
                                                  tag="tmpf")
                            nc.vector.scalar_tensor_tensor(
                                tmp_f[:], h_t[:, c, :], 1.0, w_ps[:],
                                op0=ALU.mult, op1=ALU.mult,
                                accum_out=logit_t[:, c:c + 1])
                        else:
                            w_sb = wsb_pool.tile([CHUNK, D], bf16)
                            nc.scalar.copy(w_sb[:], w_ps[:])
                            tmp_t = tmp_pool.tile([CHUNK, D], bf16)
                            nc.vector.scalar_tensor_tensor(
                                tmp_t[:], h_t[:, c, :], 1.0, w_sb[:],
                                op0=ALU.mult, op1=ALU.mult,
                                accum_out=logit_t[:, c:c + 1])
                return logit_t

            def emit_exp(m, logit_t):
                """batched exp on ACT (issued early so the later DVE
                scatter never head-of-line blocks the DVE queue)."""
                e_t = poly_pool.tile([CHUNK, CPM], f32, tag="et")
                nc.scalar.activation(
                    e_t[:], logit_t[:], AF.Exp, bias=0.0,
                    scale=(1.0 / KEY_SCALE) if fp8_oh else 1.0)
                return e_t

            def emit_tail(m, h_t, logit_t, e_t):
                """mask-scatter + pooling + normalize + out for m."""
                E = E_tiles[m % 2]
                mc = m * CPM
                if exp_mode == "act":
                    nc.vector.scalar_tensor_tensor(
                        E[0:half, 0:ECOLS:34], e_t[0:half, :], 0.0,
                        mask_t[0:half, mc:mc + CPM],
                        op0=ALU.add, op1=ALU.mult)
                    nc.vector.scalar_tensor_tensor(
                        E[half:CHUNK, 1:ECOLS:34], e_t[half:CHUNK, :],
                        0.0, mask_t[half:CHUNK, mc:mc + CPM],
                        op0=ALU.add, op1=ALU.mult)
                else:
                    deg = len(EXP_POLY) - 1
                    t_a = poly_pool.tile([CHUNK, CPM], f32, tag="ta")
                    t_b = poly_pool.tile([CHUNK, CPM], f32, tag="tb")
                    nc.vector.tensor_scalar(t_a[:], logit_t[:],
                                            EXP_POLY[deg],
                                            None, op0=ALU.mult)
                    cur, nxt = t_a, t_b
                    for k in range(deg - 1, 0, -1):
                        nc.vector.scalar_tensor_tensor(
                            nxt[:], cur[:], EXP_POLY[k], logit_t[:],
                            op0=ALU.add, op1=ALU.mult)
                        cur, nxt = nxt, cur
                    nc.vector.scalar_tensor_tensor(
                        E[0:half, 0:ECOLS:34], cur[0:half, :], EXP_POLY[0],
                        mask_t[0:half, mc:mc + CPM],
                        op0=ALU.add, op1=ALU.mult)
                    nc.vector.scalar_tensor_tensor(
                        E[half:CHUNK, 1:ECOLS:34], cur[half:CHUNK, :],
                        EXP_POLY[0], mask_t[half:CHUNK, mc:mc + CPM],
                        op0=ALU.add, op1=ALU.mult)

                # pool += E_c.T @ h_c ; esum += E_c.T @ ones
                pool_ps = psp.tile([RPM, D], f32)
                esum_ps = pse.tile([RPM, 2], f32)
                for c in range(CPM):
                    first, last = (c == 0), (c == CPM - 1)
                    Ec = E[:, 32 * c:32 * c + 32]
                    nc.tensor.matmul(pool_ps[:], Ec, h_t[:, c, :],
                                     start=first, stop=last)
                    nc.tensor.matmul(esum_ps[:], Ec, ones_t[:],
                                     start=first, stop=last)

                recip_t = recip_pool.tile([RPM, 1], f32)
                nc.vector.reciprocal(recip_t[:], esum_ps[:, 0:1])
                out_t = out_pool.tile([RPM, D], f32)
                if norm_mode == "act":
                    nc.scalar.activation(out_t[:], pool_ps[:], AF.Copy,
                                         bias=0.0, scale=recip_t[:])
                else:
                    nc.vector.tensor_scalar_mul(out_t[:], pool_ps[:],
                                                recip_t[:])
                nc.gpsimd.dma_start(out_d[m * RPM:(m + 1) * RPM, :],
                                    out_t[:])

            rep_ctx = (tc.For_i(0, repeat, 1) if repeat > 1
                       else contextlib.nullcontext())
            with rep_ctx:
                if dma_only:
                    for m in range(n_macros):
                        h_t = h_pool.tile([CHUNK, CPM, D], bf16)
                        hc = CPM // 2
                        nc.sync.dma_start(h_t[:, 0:hc, :],
                                          hd3[m][:, 0:hc, :])
                        nc.sync.dma_start(h_t[:, hc:, :], hd3[m][:, hc:, :])
                        out_t = out_pool.tile([RPM, D], f32)
                        nc.vector.tensor_copy(out_t[:], h_t[0:RPM, 0, :])
                        nc.scalar.dma_start(
                            out_d[m * RPM:(m + 1) * RPM, :], out_t[:])
                else:
                    # software pipeline: per macro, emit exp(m) first on
                    # ACT, then macro m+1's dots, then m's scatter/pooling
                    # tail — neither the DVE nor the ACT FIFO head-of-line
                    # blocks on a cross-engine dependency
                    tiles = {0: emit_dma(0)}
                    if n_macros > 1:
                        tiles[1] = emit_dma(1)
                    pending = (tiles[0][0], emit_head(0, *tiles.pop(0)))
                    for m in range(n_macros):
                        e_t = (emit_exp(m, pending[1])
                               if exp_mode == "act" else None)
                        if m + 2 < n_macros:
                            tiles[m + 2] = emit_dma(m + 2)
                        nxt = None
                        if m + 1 < n_macros:
                            tm = tiles.pop(m + 1)
                            nxt = (tm[0], emit_head(m + 1, *tm))
                        emit_tail(m, *pending, e_t)
                        pending = nxt

    nc.compile()
    return nc


# Current best configuration (kernel() and the harness use this).
DEFAULT_CFG = dict(n_f32_dots=4, exp_mode="act", norm_mode="act",
                   pe_pairs=True, fp8_oh=True)


def get_program(n_macros: int = B_CORE * K // MACRO_BK, **cfg):
    merged = {**DEFAULT_CFG, **cfg}
    key = (n_macros, tuple(sorted(merged.items())))
    if key not in _prog_cache:
        _prog_cache[key] = _build_program(n_macros, **merged)
    return _prog_cache[key]


def prep_core_inputs(h_bk: np.ndarray, sigma_bk: np.ndarray,
                     keys_bf16: np.ndarray, pe_pairs: bool = False,
                     fp8_oh: bool = False):
    """Host-side prep of one core's input map.
    h_bk: (bk, D) float32, sigma_bk: (bk,) int."""
    bk = h_bk.shape[0]
    n_macros = bk // MACRO_BK
    CPM = CHUNKS_PER_MACRO
    sig = sigma_bk.astype(np.int64)
    ohdt = ml_dtypes.float8_e4m3 if fp8_oh else ml_dtypes.bfloat16
    # one-hot (a == sigma); sigma == N_AGENTS (unassigned) matches nothing
    oh = (sig[None, :] == np.arange(N_AGENTS, dtype=np.int64)[:, None])
    oh = oh.astype(ohdt)                        # (A, bk)
    oh = oh.reshape(N_AGENTS, n_macros, CPM, CHUNK)
    oh = np.ascontiguousarray(oh.transpose(1, 0, 2, 3))  # (m, A, c, j)
    if fp8_oh:
        keys_bf16 = (keys_bf16.astype(np.float32) * KEY_SCALE).astype(ohdt)
    else:
        keys_bf16 = keys_bf16.astype(ohdt)
    if pe_pairs:
        # pack chunk pair (2i, 2i+1) into partitions 0:64 / 64:128
        ohp = np.empty((n_macros, CHUNK, CPM // 2, CHUNK),
                       dtype=ohdt)
        ohp[:, 0:N_AGENTS] = oh[:, :, 0::2, :]
        ohp[:, N_AGENTS:] = oh[:, :, 1::2, :]
        oh = ohp
        keys_bf16 = np.concatenate([keys_bf16, keys_bf16], axis=0)

    mask = (sig < N_AGENTS)
    mask = mask.reshape(n_macros, CPM, CHUNK)
    mask = np.ascontiguousarray(mask.transpose(2, 0, 1))
    mask = mask.reshape(CHUNK, n_macros * CPM).astype(ml_dtypes.bfloat16)

    h_shuf = np.ascontiguousarray(
        h_bk.reshape(n_macros, CPM, CHUNK, D)
            .transpose(0, 2, 1, 3)).astype(ml_dtypes.bfloat16)
    h_shuf = h_shuf.reshape(n_macros, CHUNK, CPM * D)

    return {
        "h": h_shuf,
        "oh": oh,
        "mask": mask,
        "keys": keys_bf16,
        "ones": np.ones((CHUNK, 2), dtype=ml_dtypes.bfloat16),
        "ez": np.zeros((CHUNK, 512), dtype=ml_dtypes.bfloat16),
    }


LAST_EXEC_NS = None
LAST_TRACE = None


def kernel(h, keys, sigma):
    global LAST_EXEC_NS, LAST_TRACE
    from concourse.bass_utils import run_bass_kernel_spmd

    h = np.asarray(h, dtype=np.float32)
    keys = np.asarray(keys, dtype=np.float32)
    sigma = np.asarray(sigma)

    keys_bf16 = keys.astype(ml_dtypes.bfloat16)
    h2 = h.reshape(B * K, D)
    sig2 = sigma.reshape(B * K)

    in_maps = []
    for i in range(N_CORES):
        lo, hi = i * BK_CORE, (i + 1) * BK_CORE
        in_maps.append(prep_core_inputs(h2[lo:hi], sig2[lo:hi], keys_bf16,
                                        pe_pairs=DEFAULT_CFG["pe_pairs"],
                                        fp8_oh=DEFAULT_CFG["fp8_oh"]))

    nc = get_program()
    res = run_bass_kernel_spmd(nc, in_maps, list(range(N_CORES)))
    out = np.concatenate([res.results[i]["out"] for i in range(N_CORES)],
                         axis=0)
    if res.exec_time_ns is not None:
        LAST_EXEC_NS = res.exec_time_ns
        LAST_TRACE = res.instructions_and_trace
    return out.astype(np.float32)


if __name__ == "__main__":
    rng = np.random.default_rng(0)
    h = rng.standard_normal((B, K, D), dtype=np.float32)
    keys = (rng.standard_normal((N_AGENTS, D), dtype=np.float32) * 0.01)
    sigma = rng.integers(0, N_AGENTS + 1, size=(B, K)).astype(np.int32)
    out = kernel(h=h, keys=keys, sigma=sigma)
    print("out", out.shape, out.dtype, float(np.abs(out).mean()))
